# revision 23
# baseline (speedup 1.0000x reference)
"""Distributed Trainium2 kernel for AssociativeSparseDistributedMemory.get_cliques.

Reference (B=128, INPUT=1024, VCAP=32768, K=32, ACAP=4096, K2=32):
  scores  = keys @ value_proj.T;  idx1 = top_k(scores, 32)
  p       = clique_encoder[idx1].sum(1)   (scale+normalize skipped: a positive
                                           per-row scale never changes a top-k set)
  scores2 = p @ assoc_proj.T;     idx2 = top_k(scores2, 32)
  out     = assoc_mem_value[idx2].sum(1)

Distribution over 8 cores (core m):
  B : value_proj rows [4096m, 4096(m+1)) -> score chunk [128, 4096], 512
      columns at a time, chunk DMAs split across both HWDGE queues; per-chunk
      top-32 values+positions on DVE overlap the next chunk's matmul.  Mb
      (stage Q rhs) prefetch DMAs ride the leftover DMA bandwidth.
  C : paired (vals | global idx) candidate lists AllGather in two waves:
      chunks 0-5 fly while chunks 6-7 still compute, then chunks 6-7.
      Each core merges all 8*256 pairs: t32 = 32nd value, mask >= t32,
      top-32-by-index of masked indices -> exact global top-32 idx, aligned
      pairing needs no second collective.
  E : indices -> int16 DGE wrapped layout (matmul with a mod-16 replicator);
      4x dma_gather on 4 SWDGE queues pulls the 4096 selected rows of the
      column-sharded clique_encoder (E[:, 512m:512(m+1)], 2KB rows); tree-sum
      over the 32 slots -> p chunk [128, 512]; AllGather p (PE-transposed
      first so the gathered result is the stage-K lhsT layout).
  K : scores2 chunk = p @ assoc_proj[512m:512(m+1)].T (fp32), apT rhs ring
      pre-issued during the gather.
  L : local top-32 values, AllGather, merge -> t32_2; mask2 = s2 >= t32_2;
      AllGather mask2 -> full selection w2 [128, 4096].
  Q : out chunk = w2 @ M[:, 4096m:4096(m+1)) in BF16 (selection already done;
      0/1 weights exact in bf16, table quantization well under tolerance).
      First NPRE k-slots come from the SBUF prefetch pool; the rest stream on
      both queues; each PSUM bank is copied + stored the moment it completes.
"""

import numpy as np

B = 128
INPUT = 1024
VCAP = 32768
ACAP = 4096
K = 32
NCORES = 8
VSH = VCAP // NCORES      # 4096 value rows per core
ASH = ACAP // NCORES      # 512 assoc rows per core
NPRE = 13                 # Mb k-slots prefetched to SBUF (of 32)

_CACHE = {}

NEG = -1e30


def _build():
    import concourse.bass as bass
    import concourse.mybir as mybir
    import concourse.tile as tile
    from concourse import bacc
    from concourse.masks import make_identity

    f32 = mybir.dt.float32
    bf16 = mybir.dt.bfloat16
    i16 = mybir.dt.int16
    u16 = mybir.dt.uint16
    u8 = mybir.dt.uint8
    Alu = mybir.AluOpType

    nc = bacc.Bacc("TRN2", target_bir_lowering=False, debug=False,
                   num_devices=NCORES)

    # ---- kernel I/O ----
    keysTt_d = nc.dram_tensor("keysTt", [128, 8, 128], f32, kind="ExternalInput")
    vpTt_d = nc.dram_tensor("vpTt", [8, 128, 8, 512], f32, kind="ExternalInput")
    Ecol_d = nc.dram_tensor("Ecol", [VCAP, ASH], f32, kind="ExternalInput")
    apT_d = nc.dram_tensor("apT", [128, 32, ASH], f32, kind="ExternalInput")
    Mb_d = nc.dram_tensor("Mb", [ACAP, VSH], bf16, kind="ExternalInput")
    rbase_d = nc.dram_tensor("rbase", [B, 1], f32, kind="ExternalInput")
    repl16_d = nc.dram_tensor("repl16", [128, 128], f32, kind="ExternalInput")
    dsel_d = nc.dram_tensor("dsel", [128, 8], f32, kind="ExternalInput")
    out_d = nc.dram_tensor("out", [B, VSH], f32, kind="ExternalOutput")

    # ---- internal DRAM ----
    cand1_in = nc.dram_tensor("cand1_in", [B, K], f32)
    cand1_out = nc.dram_tensor("cand1_out", [B * NCORES, K], f32,
                               addr_space="Shared")
    idxag_in = nc.dram_tensor("idxag_in", [B, K], f32)
    idxag_out = nc.dram_tensor("idxag_out", [B * NCORES, K], f32,
                               addr_space="Shared")
    pag_in = nc.dram_tensor("pag_in", [ASH, B], f32)
    pag_out = nc.dram_tensor("pag_out", [ASH * NCORES, B], f32,
                             addr_space="Shared")
    cand2_in = nc.dram_tensor("cand2_in", [B, K], f32)
    cand2_out = nc.dram_tensor("cand2_out", [B * NCORES, K], f32,
                               addr_space="Shared")
    m2_in = nc.dram_tensor("m2_in", [ASH, B], bf16)
    m2_out = nc.dram_tensor("m2_out", [ASH * NCORES, B], bf16,
                            addr_space="Shared")

    RG = [list(range(NCORES))]

    with tile.TileContext(nc) as tc:
        with (
            tc.tile_pool(name="const", bufs=1) as constp,
            tc.tile_pool(name="small", bufs=1) as smallp,
            tc.tile_pool(name="mbp", bufs=1) as mbp,
        ):
            psA_cm = tc.tile_pool(name="psA", bufs=2, space="PSUM")
            psA = psA_cm.__enter__()

            # ---- startup: keys + first score chunk first, consts after ----
            keysT_sb = constp.tile([128, 8, 128], f32)
            nc.sync.dma_start(out=keysT_sb[:, :, :], in_=keysTt_d[:, :, :])

            rbase = constp.tile([B, 1], f32)
            nc.scalar.dma_start(out=rbase[:, :], in_=rbase_d[:, :])
            repl16 = constp.tile([128, 128], f32)
            nc.scalar.dma_start(out=repl16[:, :], in_=repl16_d[:, :])
            dsel = constp.tile([128, 8], f32)
            nc.scalar.dma_start(out=dsel[:, :], in_=dsel_d[:, :])
            ident = constp.tile([128, 128], f32)
            make_identity(nc, ident[:, :])

            # Mb prefetch pool: NPRE persistent k-slot tiles
            mbpre = [mbp.tile([128, VSH], bf16, tag=f"mb{k}", name=f"mb{k}")
                     for k in range(NPRE)]

            # ---- stage B: score chunks + pipelined per-chunk top-32 ----
            rhsBp_cm = tc.tile_pool(name="rhsB", bufs=3)
            rhsBp = rhsBp_cm.__enter__()
            chkp_cm = tc.tile_pool(name="chk", bufs=3)
            chkp = chkp_cm.__enter__()
            # combined candidate tile: [:, 0] = values, [:, 1] = global idx
            vc = smallp.tile([B, 2, 8, K], f32)
            for n in range(8):
                ps = psA.tile([128, 512], f32, tag="ps", name=f"psB{n}")
                rhs = rhsBp.tile([128, 8, 512], f32, tag="rhs", name=f"rB{n}")
                nc.sync.dma_start(out=rhs[:, 0:4, :], in_=vpTt_d[n, :, 0:4, :])
                nc.scalar.dma_start(out=rhs[:, 4:8, :], in_=vpTt_d[n, :, 4:8, :])
                if n < 7:   # Mb prefetch rides leftover DMA bandwidth
                    nc.scalar.dma_start(out=mbpre[n][:, :],
                                        in_=Mb_d[n * 128:(n + 1) * 128, :])
                for k in range(8):
                    nc.tensor.matmul(ps[:, :], keysT_sb[:, k, :], rhs[:, k, :],
                                     start=(k == 0), stop=(k == 7))
                schunk = chkp.tile([B, 512], f32, tag="schunk", name=f"sch{n}")
                nc.scalar.copy(schunk[:, :], ps[:, :])
                scr = chkp.tile([B, 512], f32, tag="scr", name=f"scr{n}")
                idxn = chkp.tile([B, K], u16, tag="idxn", name=f"idxn{n}")
                for r in range(4):
                    s = schunk if r == 0 else scr
                    nc.vector.max(out=vc[:, 0, n, r * 8:(r + 1) * 8], in_=s[:, :])
                    nc.vector.max_index(out=idxn[:, r * 8:(r + 1) * 8],
                                        in_max=vc[:, 0, n, r * 8:(r + 1) * 8],
                                        in_values=schunk[:, :])
                    nc.vector.match_replace(
                        out=scr[:, :],
                        in_to_replace=vc[:, 0, n, r * 8:(r + 1) * 8],
                        in_values=s[:, :], imm_value=NEG)
                # global index = pos + rank_base + n*512
                nc.vector.tensor_scalar(
                    out=vc[:, 1, n, :], in0=idxn[:, :], scalar1=rbase[:, :],
                    scalar2=float(n * 512), op0=Alu.add, op1=Alu.add)
            # core-level value premerge over the 256 chunk candidates
            vals256f = vc[:, 0, :, :].rearrange("b e k -> b (e k)")
            cmv = smallp.tile([B, K], f32, tag="cmv")
            cms = smallp.tile([B, 8 * K], f32, tag="cms")
            for r in range(4):
                s = vals256f if r == 0 else cms[:, :]
                nc.vector.max(out=cmv[:, r * 8:(r + 1) * 8], in_=s)
                nc.vector.match_replace(
                    out=cms[:, :], in_to_replace=cmv[:, r * 8:(r + 1) * 8],
                    in_values=s, imm_value=NEG)
            nc.sync.dma_start(out=cand1_in[:, :], in_=cmv[:, :])
            nc.gpsimd.collective_compute(
                "AllGather", Alu.bypass, replica_groups=RG,
                ins=[cand1_in.ap().opt()], outs=[cand1_out.ap().opt()])
            chkp_cm.__exit__(None, None, None)
            rhsBp_cm.__exit__(None, None, None)

            # post-B prefetch issues: apT ring j=0,1 then remaining Mb slots
            rhsKp_cm = tc.tile_pool(name="rhsK", bufs=2)
            rhsKp = rhsKp_cm.__enter__()
            rhsK = [rhsKp.tile([128, 8, ASH], f32, tag="rhs", name=f"rK{j}")
                    for j in range(4)]
            nc.scalar.dma_start(out=rhsK[0][:, :, :], in_=apT_d[:, 0:8, :])
            nc.sync.dma_start(out=rhsK[1][:, :, :], in_=apT_d[:, 8:16, :])
            for k in range(7, NPRE):
                eng = nc.scalar if k % 2 else nc.sync
                eng.dma_start(out=mbpre[k][:, :],
                              in_=Mb_d[k * 128:(k + 1) * 128, :])

            # ---- stage C: global value merge -> t32, then index AllGather ----
            mrgp_cm = tc.tile_pool(name="mrg", bufs=1)
            mrgp = mrgp_cm.__enter__()

            def topk32(vals, width, pool, pref):
                """mv [B, 32] = top-32 values of vals [B, width] (descending)."""
                mv = pool.tile([B, K], f32, name=f"{pref}_mv", tag=f"{pref}_mv")
                ms = pool.tile([B, width], f32, name=f"{pref}_ms", tag=f"{pref}_ms")
                for r in range(4):
                    s = vals if r == 0 else ms[:, :]
                    nc.vector.max(out=mv[:, r * 8:(r + 1) * 8], in_=s)
                    nc.vector.match_replace(
                        out=ms[:, :], in_to_replace=mv[:, r * 8:(r + 1) * 8],
                        in_values=s, imm_value=NEG)
                return mv

            gvals = mrgp.tile([B, NCORES, K], f32)
            nc.sync.dma_start(
                out=gvals[:, :, :],
                in_=cand1_out.ap().rearrange("(r b) k -> b r k", r=NCORES, b=B))
            gmv = topk32(gvals[:, :, :].rearrange("b e k -> b (e k)"),
                         NCORES * K, mrgp, "gm")

            # local index extraction under the global threshold
            msk = mrgp.tile([B, 8 * K], u8)
            nc.vector.tensor_scalar(out=msk[:, :], in0=vals256f,
                                    scalar1=gmv[:, K - 1:K], scalar2=None,
                                    op0=Alu.is_ge)
            mi = mrgp.tile([B, 8 * K], f32)
            nc.vector.memset(mi[:, :], -1.0)
            nc.vector.copy_predicated(
                out=mi[:, :], mask=msk[:, :],
                data=vc[:, 1, :, :].rearrange("b e k -> b (e k)"))
            lidx = topk32(mi[:, :], 8 * K, mrgp, "li")

            nc.sync.dma_start(out=idxag_in[:, :], in_=lidx[:, :])
            nc.gpsimd.collective_compute(
                "AllGather", Alu.bypass, replica_groups=RG,
                ins=[idxag_in.ap().opt()], outs=[idxag_out.ap().opt()])
            gidxall = mrgp.tile([B, NCORES, K], f32)
            nc.sync.dma_start(
                out=gidxall[:, :, :],
                in_=idxag_out.ap().rearrange("(r b) k -> b r k", r=NCORES, b=B))
            giv = topk32(gidxall[:, :, :].rearrange("b e k -> b (e k)"),
                         NCORES * K, mrgp, "gi")

            # ---- stage E: build the DGE wrapped index layout on-chip ----
            # idxs16[p', k*8+s0] = giv[16*s0 + p'%16, k].  Spread giv
            # diagonally into R[b, k, s0] (nonzero only when b//16 == s0),
            # then one matmul with the mod-16 replicator sums it into place.
            R = smallp.tile([128, K, 8], f32)
            nc.vector.tensor_tensor(
                out=R[:, :, :],
                in0=giv[:, :].broadcast_to([128, K, 8]),
                in1=dsel[:, None, :].broadcast_to([128, K, 8]),
                op=Alu.mult)
            psI = psA.tile([128, 256], f32, tag="ps", name="psI")
            nc.tensor.matmul(psI[:, :], repl16[:, :],
                             R[:, :, :].rearrange("p k s -> p (k s)"),
                             start=True, stop=True)
            idxs16 = smallp.tile([128, 256], i16)   # 4096 idxs / 16 lanes
            nc.vector.tensor_copy(idxs16[:, :], psI[:, :])
            mrgp_cm.__exit__(None, None, None)

            # SWDGE ring holds 128 descriptors/queue; one gather emits
            # num_idxs/16+1, so split 4096 indices into 4 calls of 1024 on 4
            # queues and tree-reduce each batch of 8 slots as gathers land.
            gatp_cm = tc.tile_pool(name="gat", bufs=2)
            gatp = gatp_cm.__enter__()
            p_chunk = smallp.tile([B, ASH], f32)
            for j in range(4):
                gath = gatp.tile([128, 8, ASH], f32, tag="gath", name=f"gath{j}")
                nc.gpsimd.dma_gather(
                    out_ap=gath[:, :, :], in_ap=Ecol_d.ap(),
                    idxs_ap=idxs16[:, j * 64:(j + 1) * 64],
                    num_idxs=1024, num_idxs_reg=1024, elem_size=ASH)
                a1 = gatp.tile([B, 4, ASH], f32, tag="a1", name=f"a1_{j}", bufs=1)
                nc.vector.tensor_tensor(out=a1[:, :, :], in0=gath[:, 0:4, :],
                                        in1=gath[:, 4:8, :], op=Alu.add)
                a2 = gatp.tile([B, 2, ASH], f32, tag="a2", name=f"a2_{j}", bufs=1)
                nc.vector.tensor_tensor(out=a2[:, :, :], in0=a1[:, 0:2, :],
                                        in1=a1[:, 2:4, :], op=Alu.add)
                if j == 0:
                    nc.vector.tensor_tensor(out=p_chunk[:, :], in0=a2[:, 0, :],
                                            in1=a2[:, 1, :], op=Alu.add)
                else:
                    a3 = gatp.tile([B, ASH], f32, tag="a3", name=f"a3_{j}", bufs=1)
                    nc.vector.tensor_tensor(out=a3[:, :], in0=a2[:, 0, :],
                                            in1=a2[:, 1, :], op=Alu.add)
                    nc.vector.tensor_tensor(out=p_chunk[:, :], in0=p_chunk[:, :],
                                            in1=a3[:, :], op=Alu.add)
            gatp_cm.__exit__(None, None, None)

            # ---- transpose p chunk BEFORE the AllGather, so the gathered
            # result is directly the lhsT layout for stage K ----
            pTp_cm = tc.tile_pool(name="pTp", bufs=1)
            pTp = pTp_cm.__enter__()
            pTc = smallp.tile([128, 4, 128], f32)
            for t in range(4):
                pt = psA.tile([128, 128], f32, tag="ps", name=f"ptJ{t}")
                nc.tensor.transpose(pt[:, :], p_chunk[:, t * 128:(t + 1) * 128],
                                    ident[:, :])
                nc.scalar.copy(pTc[:, t, :], pt[:, :])
            for t in range(4):
                eng = nc.sync if t % 2 == 0 else nc.scalar
                eng.dma_start(out=pag_in[t * 128:(t + 1) * 128, :],
                              in_=pTc[:, t, :])
            nc.gpsimd.collective_compute(
                "AllGather", Alu.bypass, replica_groups=RG,
                ins=[pag_in.ap().opt()], outs=[pag_out.ap().opt()])
            pT = pTp.tile([128, 32, 128], f32)
            nc.sync.dma_start(
                out=pT[:, 0:16, :],
                in_=pag_out.ap()[0:2048, :].rearrange(
                    "(t p) c -> p t c", t=16, p=128))
            nc.scalar.dma_start(
                out=pT[:, 16:32, :],
                in_=pag_out.ap()[2048:4096, :].rearrange(
                    "(t p) c -> p t c", t=16, p=128))

            # ---- stage K: scores2 chunk (fp32) ----
            s2 = smallp.tile([B, ASH], f32, tag="s2")
            psK = psA.tile([128, 512], f32, tag="ps", name="psK")
            # ring waits stall only the issuing engine, not the matmul stream
            nc.scalar.dma_start(out=rhsK[2][:, :, :], in_=apT_d[:, 16:24, :])
            nc.sync.dma_start(out=rhsK[3][:, :, :], in_=apT_d[:, 24:32, :])
            for j in range(4):
                for k in range(8):
                    kk = j * 8 + k
                    nc.tensor.matmul(psK[:, :], pT[:, kk, :], rhsK[j][:, k, :],
                                     start=(kk == 0), stop=(kk == 31))
            nc.scalar.copy(s2[:, :], psK[:, :])
            pTp_cm.__exit__(None, None, None)
            rhsKp_cm.__exit__(None, None, None)
            bigp_cm = tc.tile_pool(name="big", bufs=1)
            bigp = bigp_cm.__enter__()

            # ---- stage L/M: local top-32 values, AG, merge -> t32_2 ----
            scr2 = smallp.tile([B, ASH], f32, tag="scr2")
            cand2 = smallp.tile([B, K], f32, tag="c2")
            for r in range(4):
                s = s2 if r == 0 else scr2
                nc.vector.max(out=cand2[:, r * 8:(r + 1) * 8], in_=s[:, :])
                nc.vector.match_replace(
                    out=scr2[:, :], in_to_replace=cand2[:, r * 8:(r + 1) * 8],
                    in_values=s[:, :], imm_value=NEG)
            nc.sync.dma_start(out=cand2_in[:, :], in_=cand2[:, :])
            nc.gpsimd.collective_compute(
                "AllGather", Alu.bypass, replica_groups=RG,
                ins=[cand2_in.ap().opt()], outs=[cand2_out.ap().opt()])
            cands2 = smallp.tile([B, NCORES, K], f32, tag="cs2")
            nc.sync.dma_start(
                out=cands2[:, :, :],
                in_=cand2_out.ap().rearrange("(r b) k -> b r k", r=NCORES, b=B))
            mcand2 = smallp.tile([B, K], f32, tag="mc2")
            mscr2 = smallp.tile([B, NCORES * K], f32, tag="ms2")
            for r in range(4):
                s = (cands2[:, :, :].rearrange("b e k -> b (e k)")
                     if r == 0 else mscr2[:, :])
                nc.vector.max(out=mcand2[:, r * 8:(r + 1) * 8], in_=s)
                nc.vector.match_replace(
                    out=mscr2[:, :], in_to_replace=mcand2[:, r * 8:(r + 1) * 8],
                    in_values=s, imm_value=NEG)

            # ---- stage N/O: mask2, AllGather -> w2 ----
            mask2 = smallp.tile([B, ASH], f32, tag="m2")
            nc.vector.tensor_scalar(
                out=mask2[:, :], in0=s2[:, :], scalar1=mcand2[:, K - 1:K],
                scalar2=None, op0=Alu.is_ge)
            m2Tc = smallp.tile([128, 4, 128], bf16)
            for t in range(4):
                pt = psA.tile([128, 128], f32, tag="ps", name=f"ptP{t}")
                nc.tensor.transpose(pt[:, :], mask2[:, t * 128:(t + 1) * 128],
                                    ident[:, :])
                nc.scalar.copy(m2Tc[:, t, :], pt[:, :])
            for t in range(4):
                eng = nc.sync if t % 2 == 0 else nc.scalar
                eng.dma_start(out=m2_in[t * 128:(t + 1) * 128, :],
                              in_=m2Tc[:, t, :])
            nc.gpsimd.collective_compute(
                "AllGather", Alu.bypass, replica_groups=RG,
                ins=[m2_in.ap().opt()], outs=[m2_out.ap().opt()])

            # ---- stage Q: out chunk = w2 @ M_shard (bf16) ----
            psA_cm.__exit__(None, None, None)
            psQp_cm = tc.tile_pool(name="psQ", bufs=8, space="PSUM")
            psQp = psQp_cm.__enter__()
            rhsQp_cm = tc.tile_pool(name="rhsQ", bufs=4)
            rhsQp = rhsQp_cm.__enter__()

            # pre-issue the first streamed rhs slots before the w2T readback
            rQ = {}
            for k in range(NPRE, min(NPRE + 4, 32)):
                rQ[k] = rhsQp.tile([128, VSH], bf16, tag="rhs", name=f"rQ{k}")
                eng = nc.sync if k % 2 == 0 else nc.scalar
                eng.dma_start(out=rQ[k][:, :],
                              in_=Mb_d[k * 128:(k + 1) * 128, :])

            w2T = bigp.tile([128, 32, 128], bf16, tag="w2T")
            nc.sync.dma_start(
                out=w2T[:, 0:16, :],
                in_=m2_out.ap()[0:2048, :].rearrange(
                    "(t p) c -> p t c", t=16, p=128))
            nc.scalar.dma_start(
                out=w2T[:, 16:32, :],
                in_=m2_out.ap()[2048:4096, :].rearrange(
                    "(t p) c -> p t c", t=16, p=128))

            out_sb = bigp.tile([B, VSH], f32, tag="B")
            psQ = [psQp.tile([128, 512], f32, tag="pq", name=f"psQ{n}")
                   for n in range(8)]
            for k in range(32):
                if k < NPRE:
                    rhs = mbpre[k]
                else:
                    if k not in rQ:
                        rQ[k] = rhsQp.tile([128, VSH], bf16, tag="rhs",
                                           name=f"rQ{k}")
                        eng = nc.sync if k % 2 == 0 else nc.scalar
                        eng.dma_start(out=rQ[k][:, :],
                                      in_=Mb_d[k * 128:(k + 1) * 128, :])
                    rhs = rQ[k]
                    if k + 4 < 32:
                        kk = k + 4
                        rQ[kk] = rhsQp.tile([128, VSH], bf16, tag="rhs",
                                            name=f"rQ{kk}")
                        eng = nc.sync if kk % 2 == 0 else nc.scalar
                        eng.dma_start(out=rQ[kk][:, :],
                                      in_=Mb_d[kk * 128:(kk + 1) * 128, :])
                for n in range(8):
                    nc.tensor.matmul(psQ[n][:, :], w2T[:, k, :],
                                     rhs[:, n * 512:(n + 1) * 512],
                                     start=(k == 0), stop=(k == 31))
            # stream each bank out as it completes
            for n in range(8):
                ceng = nc.scalar if n % 2 == 0 else nc.vector
                if n % 2 == 0:
                    ceng.copy(out_sb[:, n * 512:(n + 1) * 512], psQ[n][:, :])
                else:
                    ceng.tensor_copy(out_sb[:, n * 512:(n + 1) * 512],
                                     psQ[n][:, :])
                deng = nc.sync if n % 2 == 0 else nc.scalar
                deng.dma_start(out=out_d[:, n * 512:(n + 1) * 512],
                               in_=out_sb[:, n * 512:(n + 1) * 512])
            psQp_cm.__exit__(None, None, None)
            rhsQp_cm.__exit__(None, None, None)
            bigp_cm.__exit__(None, None, None)

    nc.compile()
    return nc


def get_nc():
    if "nc" not in _CACHE:
        _CACHE["nc"] = _build()
    return _CACHE["nc"]


def make_in_maps(keys, value_proj, clique_encoder, assoc_proj, assoc_mem_value):
    import ml_dtypes
    keysT = np.asarray(keys).T.astype(np.float32)          # [1024, 128]
    keysTt = np.ascontiguousarray(
        keysT.reshape(8, 128, 128).transpose(1, 0, 2))     # [128, 8, 128]
    value_proj = np.asarray(value_proj).astype(np.float32)
    clique_encoder = np.asarray(clique_encoder).astype(np.float32)
    assoc_proj = np.asarray(assoc_proj).astype(np.float32)
    Mb_full = np.asarray(assoc_mem_value).astype(ml_dtypes.bfloat16)
    bb, pp = np.meshgrid(np.arange(128), np.arange(128), indexing="ij")
    repl16 = (bb % 16 == pp % 16).astype(np.float32)
    dsel = (np.arange(128)[:, None] // 16 == np.arange(8)[None, :]).astype(np.float32)
    in_maps = []
    for m in range(NCORES):
        vpT = np.ascontiguousarray(
            value_proj[m * VSH:(m + 1) * VSH, :].T)        # [1024, 4096]
        # [n, p, k, c] so each n-chunk loads with one contiguous-per-partition DMA
        vpTt = np.ascontiguousarray(
            vpT.reshape(8, 128, 8, 512).transpose(2, 1, 0, 3))
        in_maps.append({
            "keysTt": keysTt,
            "vpTt": vpTt,
            "Ecol": np.ascontiguousarray(
                clique_encoder[:, m * ASH:(m + 1) * ASH]),
            "apT": np.ascontiguousarray(
                assoc_proj[m * ASH:(m + 1) * ASH, :].T
                .reshape(32, 128, ASH).transpose(1, 0, 2)),
            "Mb": np.ascontiguousarray(Mb_full[:, m * VSH:(m + 1) * VSH]),
            "rbase": np.full((B, 1), m * VSH, np.float32),
            "repl16": repl16,
            "dsel": dsel,
        })
    return in_maps


def kernel(keys, value_proj, clique_encoder, assoc_proj, assoc_mem_value,
           **run_kwargs):
    from concourse.bass_utils import run_bass_kernel_spmd

    nc = get_nc()
    in_maps = make_in_maps(keys, value_proj, clique_encoder, assoc_proj,
                           assoc_mem_value)
    res = run_bass_kernel_spmd(nc, in_maps, core_ids=list(range(NCORES)),
                               **run_kwargs)
    out = np.concatenate([np.asarray(res.results[m]["out"])
                          for m in range(NCORES)], axis=1)
    _CACHE["last_result"] = res
    return out


# revision 24
# speedup vs baseline: 1.0666x; 1.0666x over previous
"""Distributed Trainium2 kernel for AssociativeSparseDistributedMemory.get_cliques.

Reference (B=128, INPUT=1024, VCAP=32768, K=32, ACAP=4096, K2=32):
  scores  = keys @ value_proj.T;  idx1 = top_k(scores, 32)
  p       = clique_encoder[idx1].sum(1)   (scale+normalize skipped: a positive
                                           per-row scale never changes a top-k set)
  scores2 = p @ assoc_proj.T;     idx2 = top_k(scores2, 32)
  out     = assoc_mem_value[idx2].sum(1)

Distribution over 8 cores (core m):
  B : value_proj rows [4096m, 4096(m+1)) -> score chunk [128, 4096], 512
      columns at a time, chunk DMAs split across both HWDGE queues; per-chunk
      top-32 values+positions on DVE overlap the next chunk's matmul.  Mb
      (stage Q rhs) prefetch DMAs ride the leftover DMA bandwidth.
  C : paired (vals | global idx) candidate lists AllGather in two waves:
      chunks 0-5 fly while chunks 6-7 still compute, then chunks 6-7.
      Each core merges all 8*256 pairs: t32 = 32nd value, mask >= t32,
      top-32-by-index of masked indices -> exact global top-32 idx, aligned
      pairing needs no second collective.
  E : indices -> int16 DGE wrapped layout (matmul with a mod-16 replicator);
      4x dma_gather on 4 SWDGE queues pulls the 4096 selected rows of the
      column-sharded clique_encoder (E[:, 512m:512(m+1)], 2KB rows); tree-sum
      over the 32 slots -> p chunk [128, 512]; AllGather p (PE-transposed
      first so the gathered result is the stage-K lhsT layout).
  K : scores2 chunk = p @ assoc_proj[512m:512(m+1)].T (fp32), apT rhs ring
      pre-issued during the gather.
  L : local top-32 values, AllGather, merge -> t32_2; mask2 = s2 >= t32_2;
      AllGather mask2 -> full selection w2 [128, 4096].
  Q : out chunk = w2 @ M[:, 4096m:4096(m+1)) in BF16 (selection already done;
      0/1 weights exact in bf16, table quantization well under tolerance).
      First NPRE k-slots come from the SBUF prefetch pool; the rest stream on
      both queues; each PSUM bank is copied + stored the moment it completes.
"""

import numpy as np

B = 128
INPUT = 1024
VCAP = 32768
ACAP = 4096
K = 32
NCORES = 8
VSH = VCAP // NCORES      # 4096 value rows per core
ASH = ACAP // NCORES      # 512 assoc rows per core
NPRE = 13                 # Mb k-slots prefetched to SBUF (of 32)

_CACHE = {}

NEG = -1e30


def _build():
    import concourse.bass as bass
    import concourse.mybir as mybir
    import concourse.tile as tile
    from concourse import bacc
    from concourse.masks import make_identity

    f32 = mybir.dt.float32
    bf16 = mybir.dt.bfloat16
    i16 = mybir.dt.int16
    u16 = mybir.dt.uint16
    u8 = mybir.dt.uint8
    Alu = mybir.AluOpType

    nc = bacc.Bacc("TRN2", target_bir_lowering=False, debug=False,
                   num_devices=NCORES)

    # ---- kernel I/O ----
    keysTt_d = nc.dram_tensor("keysTt", [128, 8, 128], f32, kind="ExternalInput")
    vpTt_d = nc.dram_tensor("vpTt", [8, 128, 8, 512], f32, kind="ExternalInput")
    Ecol_d = nc.dram_tensor("Ecol", [VCAP, ASH], f32, kind="ExternalInput")
    apT_d = nc.dram_tensor("apT", [128, 32, ASH], f32, kind="ExternalInput")
    Mb_d = nc.dram_tensor("Mb", [ACAP, VSH], bf16, kind="ExternalInput")
    rbase_d = nc.dram_tensor("rbase", [B, 1], f32, kind="ExternalInput")
    repl16_d = nc.dram_tensor("repl16", [128, 128], f32, kind="ExternalInput")
    dsel_d = nc.dram_tensor("dsel", [128, 8], f32, kind="ExternalInput")
    out_d = nc.dram_tensor("out", [B, VSH], f32, kind="ExternalOutput")

    # ---- internal DRAM ----
    cand1_in = nc.dram_tensor("cand1_in", [B, K], f32)
    cand1_out = nc.dram_tensor("cand1_out", [B * NCORES, K], f32,
                               addr_space="Shared")
    idxag_in = nc.dram_tensor("idxag_in", [B, K], f32)
    idxag_out = nc.dram_tensor("idxag_out", [B * NCORES, K], f32,
                               addr_space="Shared")
    pag_in = nc.dram_tensor("pag_in", [ASH, B], f32)
    pag_out = nc.dram_tensor("pag_out", [ASH * NCORES, B], f32,
                             addr_space="Shared")
    cand2_in = nc.dram_tensor("cand2_in", [B, K], f32)
    cand2_out = nc.dram_tensor("cand2_out", [B * NCORES, K], f32,
                               addr_space="Shared")
    m2_in = nc.dram_tensor("m2_in", [ASH, B], bf16)
    m2_out = nc.dram_tensor("m2_out", [ASH * NCORES, B], bf16,
                            addr_space="Shared")

    RG = [list(range(NCORES))]

    with tile.TileContext(nc) as tc:
        with (
            tc.tile_pool(name="const", bufs=1) as constp,
            tc.tile_pool(name="small", bufs=1) as smallp,
            tc.tile_pool(name="mbp", bufs=1) as mbp,
        ):
            psA_cm = tc.tile_pool(name="psA", bufs=2, space="PSUM")
            psA = psA_cm.__enter__()

            # ---- startup: keys + first score chunk first, consts after ----
            keysT_sb = constp.tile([128, 8, 128], f32)
            nc.sync.dma_start(out=keysT_sb[:, :, :], in_=keysTt_d[:, :, :])

            rbase = constp.tile([B, 1], f32)
            nc.scalar.dma_start(out=rbase[:, :], in_=rbase_d[:, :])
            repl16 = constp.tile([128, 128], f32)
            nc.scalar.dma_start(out=repl16[:, :], in_=repl16_d[:, :])
            dsel = constp.tile([128, 8], f32)
            nc.scalar.dma_start(out=dsel[:, :], in_=dsel_d[:, :])
            ident = constp.tile([128, 128], f32)
            make_identity(nc, ident[:, :])

            # Mb prefetch pool: NPRE persistent k-slot tiles
            mbpre = [mbp.tile([128, VSH], bf16, tag=f"mb{k}", name=f"mb{k}")
                     for k in range(NPRE)]

            # ---- stage B: score chunks + pipelined per-chunk top-32 ----
            rhsBp_cm = tc.tile_pool(name="rhsB", bufs=3)
            rhsBp = rhsBp_cm.__enter__()
            chkp_cm = tc.tile_pool(name="chk", bufs=3)
            chkp = chkp_cm.__enter__()
            # combined candidate tile: [:, 0] = values, [:, 1] = global idx
            vc = smallp.tile([B, 2, 8, K], f32)
            for n in range(8):
                ps = psA.tile([128, 512], f32, tag="ps", name=f"psB{n}")
                rhs = rhsBp.tile([128, 8, 512], f32, tag="rhs", name=f"rB{n}")
                nc.sync.dma_start(out=rhs[:, 0:4, :], in_=vpTt_d[n, :, 0:4, :])
                nc.scalar.dma_start(out=rhs[:, 4:8, :], in_=vpTt_d[n, :, 4:8, :])
                if n < 7:   # Mb prefetch rides leftover DMA bandwidth
                    eng = nc.sync if n % 2 else nc.scalar
                    eng.dma_start(out=mbpre[n][:, :],
                                  in_=Mb_d[n * 128:(n + 1) * 128, :])
                for k in range(8):
                    nc.tensor.matmul(ps[:, :], keysT_sb[:, k, :], rhs[:, k, :],
                                     start=(k == 0), stop=(k == 7))
                # copy on vector: scalar/sync stay pure DMA issuers in B, so
                # chunk loads enqueue far ahead of the compute
                schunk = chkp.tile([B, 512], f32, tag="schunk", name=f"sch{n}")
                nc.vector.tensor_copy(schunk[:, :], ps[:, :])
                scr = chkp.tile([B, 512], f32, tag="scr", name=f"scr{n}")
                idxn = chkp.tile([B, K], u16, tag="idxn", name=f"idxn{n}")
                for r in range(4):
                    s = schunk if r == 0 else scr
                    nc.vector.max(out=vc[:, 0, n, r * 8:(r + 1) * 8], in_=s[:, :])
                    nc.vector.max_index(out=idxn[:, r * 8:(r + 1) * 8],
                                        in_max=vc[:, 0, n, r * 8:(r + 1) * 8],
                                        in_values=schunk[:, :])
                    nc.vector.match_replace(
                        out=scr[:, :],
                        in_to_replace=vc[:, 0, n, r * 8:(r + 1) * 8],
                        in_values=s[:, :], imm_value=NEG)
                # global index = pos + rank_base + n*512
                nc.vector.tensor_scalar(
                    out=vc[:, 1, n, :], in0=idxn[:, :], scalar1=rbase[:, :],
                    scalar2=float(n * 512), op0=Alu.add, op1=Alu.add)
            # core-level value premerge over the 256 chunk candidates
            vals256f = vc[:, 0, :, :].rearrange("b e k -> b (e k)")
            cmv = smallp.tile([B, K], f32, tag="cmv")
            cms = smallp.tile([B, 8 * K], f32, tag="cms")
            for r in range(4):
                s = vals256f if r == 0 else cms[:, :]
                nc.vector.max(out=cmv[:, r * 8:(r + 1) * 8], in_=s)
                nc.vector.match_replace(
                    out=cms[:, :], in_to_replace=cmv[:, r * 8:(r + 1) * 8],
                    in_values=s, imm_value=NEG)
            nc.sync.dma_start(out=cand1_in[:, :], in_=cmv[:, :])
            nc.gpsimd.collective_compute(
                "AllGather", Alu.bypass, replica_groups=RG,
                ins=[cand1_in.ap().opt()], outs=[cand1_out.ap().opt()])
            chkp_cm.__exit__(None, None, None)
            rhsBp_cm.__exit__(None, None, None)

            # post-B prefetch issues: apT ring j=0,1 then remaining Mb slots
            rhsKp_cm = tc.tile_pool(name="rhsK", bufs=2)
            rhsKp = rhsKp_cm.__enter__()
            rhsK = [rhsKp.tile([128, 8, ASH], f32, tag="rhs", name=f"rK{j}")
                    for j in range(4)]
            nc.scalar.dma_start(out=rhsK[0][:, :, :], in_=apT_d[:, 0:8, :])
            nc.sync.dma_start(out=rhsK[1][:, :, :], in_=apT_d[:, 8:16, :])
            for k in range(7, NPRE):
                eng = nc.scalar if k % 2 else nc.sync
                eng.dma_start(out=mbpre[k][:, :],
                              in_=Mb_d[k * 128:(k + 1) * 128, :])

            # ---- stage C: global value merge -> t32, then index AllGather ----
            mrgp_cm = tc.tile_pool(name="mrg", bufs=1)
            mrgp = mrgp_cm.__enter__()

            def topk32(vals, width, pool, pref):
                """mv [B, 32] = top-32 values of vals [B, width] (descending)."""
                mv = pool.tile([B, K], f32, name=f"{pref}_mv", tag=f"{pref}_mv")
                ms = pool.tile([B, width], f32, name=f"{pref}_ms", tag=f"{pref}_ms")
                for r in range(4):
                    s = vals if r == 0 else ms[:, :]
                    nc.vector.max(out=mv[:, r * 8:(r + 1) * 8], in_=s)
                    nc.vector.match_replace(
                        out=ms[:, :], in_to_replace=mv[:, r * 8:(r + 1) * 8],
                        in_values=s, imm_value=NEG)
                return mv

            gvals = mrgp.tile([B, NCORES, K], f32)
            nc.sync.dma_start(
                out=gvals[:, :, :],
                in_=cand1_out.ap().rearrange("(r b) k -> b r k", r=NCORES, b=B))
            gmv = topk32(gvals[:, :, :].rearrange("b e k -> b (e k)"),
                         NCORES * K, mrgp, "gm")

            # local index extraction under the global threshold
            msk = mrgp.tile([B, 8 * K], u8)
            nc.vector.tensor_scalar(out=msk[:, :], in0=vals256f,
                                    scalar1=gmv[:, K - 1:K], scalar2=None,
                                    op0=Alu.is_ge)
            mi = mrgp.tile([B, 8 * K], f32)
            nc.vector.memset(mi[:, :], -1.0)
            nc.vector.copy_predicated(
                out=mi[:, :], mask=msk[:, :],
                data=vc[:, 1, :, :].rearrange("b e k -> b (e k)"))
            lidx = topk32(mi[:, :], 8 * K, mrgp, "li")

            nc.sync.dma_start(out=idxag_in[:, :], in_=lidx[:, :])
            nc.gpsimd.collective_compute(
                "AllGather", Alu.bypass, replica_groups=RG,
                ins=[idxag_in.ap().opt()], outs=[idxag_out.ap().opt()])
            gidxall = mrgp.tile([B, NCORES, K], f32)
            nc.sync.dma_start(
                out=gidxall[:, :, :],
                in_=idxag_out.ap().rearrange("(r b) k -> b r k", r=NCORES, b=B))
            giv = topk32(gidxall[:, :, :].rearrange("b e k -> b (e k)"),
                         NCORES * K, mrgp, "gi")

            # ---- stage E: build the DGE wrapped index layout on-chip ----
            # idxs16[p', k*8+s0] = giv[16*s0 + p'%16, k].  Spread giv
            # diagonally into R[b, k, s0] (nonzero only when b//16 == s0),
            # then one matmul with the mod-16 replicator sums it into place.
            R = smallp.tile([128, K, 8], f32)
            nc.vector.tensor_tensor(
                out=R[:, :, :],
                in0=giv[:, :].broadcast_to([128, K, 8]),
                in1=dsel[:, None, :].broadcast_to([128, K, 8]),
                op=Alu.mult)
            psI = psA.tile([128, 256], f32, tag="ps", name="psI")
            nc.tensor.matmul(psI[:, :], repl16[:, :],
                             R[:, :, :].rearrange("p k s -> p (k s)"),
                             start=True, stop=True)
            idxs16 = smallp.tile([128, 256], i16)   # 4096 idxs / 16 lanes
            nc.vector.tensor_copy(idxs16[:, :], psI[:, :])
            mrgp_cm.__exit__(None, None, None)

            # SWDGE ring holds 128 descriptors/queue; one gather emits
            # num_idxs/16+1, so split 4096 indices into 4 calls of 1024 on 4
            # queues and tree-reduce each batch of 8 slots as gathers land.
            gatp_cm = tc.tile_pool(name="gat", bufs=2)
            gatp = gatp_cm.__enter__()
            p_chunk = smallp.tile([B, ASH], f32)
            for j in range(4):
                gath = gatp.tile([128, 8, ASH], f32, tag="gath", name=f"gath{j}")
                nc.gpsimd.dma_gather(
                    out_ap=gath[:, :, :], in_ap=Ecol_d.ap(),
                    idxs_ap=idxs16[:, j * 64:(j + 1) * 64],
                    num_idxs=1024, num_idxs_reg=1024, elem_size=ASH)
                a1 = gatp.tile([B, 4, ASH], f32, tag="a1", name=f"a1_{j}", bufs=1)
                nc.vector.tensor_tensor(out=a1[:, :, :], in0=gath[:, 0:4, :],
                                        in1=gath[:, 4:8, :], op=Alu.add)
                a2 = gatp.tile([B, 2, ASH], f32, tag="a2", name=f"a2_{j}", bufs=1)
                nc.vector.tensor_tensor(out=a2[:, :, :], in0=a1[:, 0:2, :],
                                        in1=a1[:, 2:4, :], op=Alu.add)
                if j == 0:
                    nc.vector.tensor_tensor(out=p_chunk[:, :], in0=a2[:, 0, :],
                                            in1=a2[:, 1, :], op=Alu.add)
                else:
                    a3 = gatp.tile([B, ASH], f32, tag="a3", name=f"a3_{j}", bufs=1)
                    nc.vector.tensor_tensor(out=a3[:, :], in0=a2[:, 0, :],
                                            in1=a2[:, 1, :], op=Alu.add)
                    nc.vector.tensor_tensor(out=p_chunk[:, :], in0=p_chunk[:, :],
                                            in1=a3[:, :], op=Alu.add)
            gatp_cm.__exit__(None, None, None)

            # ---- transpose p chunk BEFORE the AllGather, so the gathered
            # result is directly the lhsT layout for stage K ----
            pTp_cm = tc.tile_pool(name="pTp", bufs=1)
            pTp = pTp_cm.__enter__()
            pTc = smallp.tile([128, 4, 128], f32)
            for t in range(4):
                pt = psA.tile([128, 128], f32, tag="ps", name=f"ptJ{t}")
                nc.tensor.transpose(pt[:, :], p_chunk[:, t * 128:(t + 1) * 128],
                                    ident[:, :])
                nc.scalar.copy(pTc[:, t, :], pt[:, :])
            for t in range(4):
                eng = nc.sync if t % 2 == 0 else nc.scalar
                eng.dma_start(out=pag_in[t * 128:(t + 1) * 128, :],
                              in_=pTc[:, t, :])
            nc.gpsimd.collective_compute(
                "AllGather", Alu.bypass, replica_groups=RG,
                ins=[pag_in.ap().opt()], outs=[pag_out.ap().opt()])
            pT = pTp.tile([128, 32, 128], f32)
            nc.sync.dma_start(
                out=pT[:, 0:16, :],
                in_=pag_out.ap()[0:2048, :].rearrange(
                    "(t p) c -> p t c", t=16, p=128))
            nc.scalar.dma_start(
                out=pT[:, 16:32, :],
                in_=pag_out.ap()[2048:4096, :].rearrange(
                    "(t p) c -> p t c", t=16, p=128))

            # ---- stage K: scores2 chunk (fp32) ----
            s2 = smallp.tile([B, ASH], f32, tag="s2")
            psK = psA.tile([128, 512], f32, tag="ps", name="psK")
            # ring waits stall only the issuing engine, not the matmul stream
            nc.scalar.dma_start(out=rhsK[2][:, :, :], in_=apT_d[:, 16:24, :])
            nc.sync.dma_start(out=rhsK[3][:, :, :], in_=apT_d[:, 24:32, :])
            for j in range(4):
                for k in range(8):
                    kk = j * 8 + k
                    nc.tensor.matmul(psK[:, :], pT[:, kk, :], rhsK[j][:, k, :],
                                     start=(kk == 0), stop=(kk == 31))
            nc.scalar.copy(s2[:, :], psK[:, :])
            pTp_cm.__exit__(None, None, None)
            rhsKp_cm.__exit__(None, None, None)
            bigp_cm = tc.tile_pool(name="big", bufs=1)
            bigp = bigp_cm.__enter__()

            # ---- stage L/M: local top-32 values, AG, merge -> t32_2 ----
            scr2 = smallp.tile([B, ASH], f32, tag="scr2")
            cand2 = smallp.tile([B, K], f32, tag="c2")
            for r in range(4):
                s = s2 if r == 0 else scr2
                nc.vector.max(out=cand2[:, r * 8:(r + 1) * 8], in_=s[:, :])
                nc.vector.match_replace(
                    out=scr2[:, :], in_to_replace=cand2[:, r * 8:(r + 1) * 8],
                    in_values=s[:, :], imm_value=NEG)
            nc.sync.dma_start(out=cand2_in[:, :], in_=cand2[:, :])
            nc.gpsimd.collective_compute(
                "AllGather", Alu.bypass, replica_groups=RG,
                ins=[cand2_in.ap().opt()], outs=[cand2_out.ap().opt()])
            cands2 = smallp.tile([B, NCORES, K], f32, tag="cs2")
            nc.sync.dma_start(
                out=cands2[:, :, :],
                in_=cand2_out.ap().rearrange("(r b) k -> b r k", r=NCORES, b=B))
            mcand2 = smallp.tile([B, K], f32, tag="mc2")
            mscr2 = smallp.tile([B, NCORES * K], f32, tag="ms2")
            for r in range(4):
                s = (cands2[:, :, :].rearrange("b e k -> b (e k)")
                     if r == 0 else mscr2[:, :])
                nc.vector.max(out=mcand2[:, r * 8:(r + 1) * 8], in_=s)
                nc.vector.match_replace(
                    out=mscr2[:, :], in_to_replace=mcand2[:, r * 8:(r + 1) * 8],
                    in_values=s, imm_value=NEG)

            # ---- stage N/O: mask2, AllGather -> w2 ----
            mask2 = smallp.tile([B, ASH], f32, tag="m2")
            nc.vector.tensor_scalar(
                out=mask2[:, :], in0=s2[:, :], scalar1=mcand2[:, K - 1:K],
                scalar2=None, op0=Alu.is_ge)
            m2Tc = smallp.tile([128, 4, 128], bf16)
            for t in range(4):
                pt = psA.tile([128, 128], f32, tag="ps", name=f"ptP{t}")
                nc.tensor.transpose(pt[:, :], mask2[:, t * 128:(t + 1) * 128],
                                    ident[:, :])
                nc.scalar.copy(m2Tc[:, t, :], pt[:, :])
            for t in range(4):
                eng = nc.sync if t % 2 == 0 else nc.scalar
                eng.dma_start(out=m2_in[t * 128:(t + 1) * 128, :],
                              in_=m2Tc[:, t, :])
            nc.gpsimd.collective_compute(
                "AllGather", Alu.bypass, replica_groups=RG,
                ins=[m2_in.ap().opt()], outs=[m2_out.ap().opt()])

            # ---- stage Q: out chunk = w2 @ M_shard (bf16) ----
            psA_cm.__exit__(None, None, None)
            psQp_cm = tc.tile_pool(name="psQ", bufs=8, space="PSUM")
            psQp = psQp_cm.__enter__()
            rhsQp_cm = tc.tile_pool(name="rhsQ", bufs=4)
            rhsQp = rhsQp_cm.__enter__()

            # pre-issue the first streamed rhs slots before the w2T readback
            rQ = {}
            for k in range(NPRE, min(NPRE + 4, 32)):
                rQ[k] = rhsQp.tile([128, VSH], bf16, tag="rhs", name=f"rQ{k}")
                eng = nc.sync if k % 2 == 0 else nc.scalar
                eng.dma_start(out=rQ[k][:, :],
                              in_=Mb_d[k * 128:(k + 1) * 128, :])

            w2T = bigp.tile([128, 32, 128], bf16, tag="w2T")
            nc.sync.dma_start(
                out=w2T[:, 0:16, :],
                in_=m2_out.ap()[0:2048, :].rearrange(
                    "(t p) c -> p t c", t=16, p=128))
            nc.scalar.dma_start(
                out=w2T[:, 16:32, :],
                in_=m2_out.ap()[2048:4096, :].rearrange(
                    "(t p) c -> p t c", t=16, p=128))

            out_sb = bigp.tile([B, VSH], f32, tag="B")
            psQ = [psQp.tile([128, 512], f32, tag="pq", name=f"psQ{n}")
                   for n in range(8)]
            for k in range(32):
                if k < NPRE:
                    rhs = mbpre[k]
                else:
                    if k not in rQ:
                        rQ[k] = rhsQp.tile([128, VSH], bf16, tag="rhs",
                                           name=f"rQ{k}")
                        eng = nc.sync if k % 2 == 0 else nc.scalar
                        eng.dma_start(out=rQ[k][:, :],
                                      in_=Mb_d[k * 128:(k + 1) * 128, :])
                    rhs = rQ[k]
                    if k + 4 < 32:
                        kk = k + 4
                        rQ[kk] = rhsQp.tile([128, VSH], bf16, tag="rhs",
                                            name=f"rQ{kk}")
                        eng = nc.sync if kk % 2 == 0 else nc.scalar
                        eng.dma_start(out=rQ[kk][:, :],
                                      in_=Mb_d[kk * 128:(kk + 1) * 128, :])
                for n in range(8):
                    nc.tensor.matmul(psQ[n][:, :], w2T[:, k, :],
                                     rhs[:, n * 512:(n + 1) * 512],
                                     start=(k == 0), stop=(k == 31))
            # stream each bank out as it completes
            for n in range(8):
                ceng = nc.scalar if n % 2 == 0 else nc.vector
                if n % 2 == 0:
                    ceng.copy(out_sb[:, n * 512:(n + 1) * 512], psQ[n][:, :])
                else:
                    ceng.tensor_copy(out_sb[:, n * 512:(n + 1) * 512],
                                     psQ[n][:, :])
                deng = nc.sync if n % 2 == 0 else nc.scalar
                deng.dma_start(out=out_d[:, n * 512:(n + 1) * 512],
                               in_=out_sb[:, n * 512:(n + 1) * 512])
            psQp_cm.__exit__(None, None, None)
            rhsQp_cm.__exit__(None, None, None)
            bigp_cm.__exit__(None, None, None)

    nc.compile()
    return nc


def get_nc():
    if "nc" not in _CACHE:
        _CACHE["nc"] = _build()
    return _CACHE["nc"]


def make_in_maps(keys, value_proj, clique_encoder, assoc_proj, assoc_mem_value):
    import ml_dtypes
    keysT = np.asarray(keys).T.astype(np.float32)          # [1024, 128]
    keysTt = np.ascontiguousarray(
        keysT.reshape(8, 128, 128).transpose(1, 0, 2))     # [128, 8, 128]
    value_proj = np.asarray(value_proj).astype(np.float32)
    clique_encoder = np.asarray(clique_encoder).astype(np.float32)
    assoc_proj = np.asarray(assoc_proj).astype(np.float32)
    Mb_full = np.asarray(assoc_mem_value).astype(ml_dtypes.bfloat16)
    bb, pp = np.meshgrid(np.arange(128), np.arange(128), indexing="ij")
    repl16 = (bb % 16 == pp % 16).astype(np.float32)
    dsel = (np.arange(128)[:, None] // 16 == np.arange(8)[None, :]).astype(np.float32)
    in_maps = []
    for m in range(NCORES):
        vpT = np.ascontiguousarray(
            value_proj[m * VSH:(m + 1) * VSH, :].T)        # [1024, 4096]
        # [n, p, k, c] so each n-chunk loads with one contiguous-per-partition DMA
        vpTt = np.ascontiguousarray(
            vpT.reshape(8, 128, 8, 512).transpose(2, 1, 0, 3))
        in_maps.append({
            "keysTt": keysTt,
            "vpTt": vpTt,
            "Ecol": np.ascontiguousarray(
                clique_encoder[:, m * ASH:(m + 1) * ASH]),
            "apT": np.ascontiguousarray(
                assoc_proj[m * ASH:(m + 1) * ASH, :].T
                .reshape(32, 128, ASH).transpose(1, 0, 2)),
            "Mb": np.ascontiguousarray(Mb_full[:, m * VSH:(m + 1) * VSH]),
            "rbase": np.full((B, 1), m * VSH, np.float32),
            "repl16": repl16,
            "dsel": dsel,
        })
    return in_maps


def kernel(keys, value_proj, clique_encoder, assoc_proj, assoc_mem_value,
           **run_kwargs):
    from concourse.bass_utils import run_bass_kernel_spmd

    nc = get_nc()
    in_maps = make_in_maps(keys, value_proj, clique_encoder, assoc_proj,
                           assoc_mem_value)
    res = run_bass_kernel_spmd(nc, in_maps, core_ids=list(range(NCORES)),
                               **run_kwargs)
    out = np.concatenate([np.asarray(res.results[m]["out"])
                          for m in range(NCORES)], axis=1)
    _CACHE["last_result"] = res
    return out


# revision 30
# speedup vs baseline: 1.1563x; 1.0841x over previous
"""Distributed Trainium2 kernel for AssociativeSparseDistributedMemory.get_cliques.

Reference (B=128, INPUT=1024, VCAP=32768, K=32, ACAP=4096, K2=32):
  scores  = keys @ value_proj.T;  idx1 = top_k(scores, 32)
  p       = clique_encoder[idx1].sum(1)   (scale+normalize skipped: a positive
                                           per-row scale never changes a top-k set)
  scores2 = p @ assoc_proj.T;     idx2 = top_k(scores2, 32)
  out     = assoc_mem_value[idx2].sum(1)

Distribution over 8 cores (core m):
  B : value_proj rows [4096m, 4096(m+1)) -> score chunk [128, 4096], 512
      columns at a time, chunk DMAs split across both HWDGE queues; per-chunk
      top-32 values+positions on DVE overlap the next chunk's matmul.  Mb
      (stage Q rhs) prefetch DMAs ride the leftover DMA bandwidth.
  C : paired (vals | global idx) candidate lists AllGather in two waves:
      chunks 0-5 fly while chunks 6-7 still compute, then chunks 6-7.
      Each core merges all 8*256 pairs: t32 = 32nd value, mask >= t32,
      top-32-by-index of masked indices -> exact global top-32 idx, aligned
      pairing needs no second collective.
  E : indices -> int16 DGE wrapped layout (matmul with a mod-16 replicator);
      4x dma_gather on 4 SWDGE queues pulls the 4096 selected rows of the
      column-sharded clique_encoder (E[:, 512m:512(m+1)], 2KB rows); tree-sum
      over the 32 slots -> p chunk [128, 512]; AllGather p (PE-transposed
      first so the gathered result is the stage-K lhsT layout).
  K : scores2 chunk = p @ assoc_proj[512m:512(m+1)].T (fp32), apT rhs ring
      pre-issued during the gather.
  L : local top-32 values, AllGather, merge -> t32_2; mask2 = s2 >= t32_2;
      AllGather mask2 -> full selection w2 [128, 4096].
  Q : out chunk = w2 @ M[:, 4096m:4096(m+1)) in BF16 (selection already done;
      0/1 weights exact in bf16, table quantization well under tolerance).
      First NPRE k-slots come from the SBUF prefetch pool; the rest stream on
      both queues; each PSUM bank is copied + stored the moment it completes.
"""

import numpy as np

B = 128
INPUT = 1024
VCAP = 32768
ACAP = 4096
K = 32
NCORES = 8
VSH = VCAP // NCORES      # 4096 value rows per core
ASH = ACAP // NCORES      # 512 assoc rows per core
NPRE = 13                 # Mb k-slots prefetched to SBUF (of 32)

_CACHE = {}

NEG = -1e30


def _build():
    import concourse.bass as bass
    import concourse.mybir as mybir
    import concourse.tile as tile
    from concourse import bacc
    from concourse.masks import make_identity

    f32 = mybir.dt.float32
    bf16 = mybir.dt.bfloat16
    i16 = mybir.dt.int16
    u16 = mybir.dt.uint16
    u8 = mybir.dt.uint8
    Alu = mybir.AluOpType

    nc = bacc.Bacc("TRN2", target_bir_lowering=False, debug=False,
                   num_devices=NCORES)

    # ---- kernel I/O ----
    keysTt_d = nc.dram_tensor("keysTt", [128, 8, 128], f32, kind="ExternalInput")
    vpTt_d = nc.dram_tensor("vpTt", [8, 128, 8, 512], f32, kind="ExternalInput")
    Ecol_d = nc.dram_tensor("Ecol", [VCAP, ASH], f32, kind="ExternalInput")
    apT_d = nc.dram_tensor("apT", [128, 32, ASH], f32, kind="ExternalInput")
    Mb_d = nc.dram_tensor("Mb", [ACAP, VSH], bf16, kind="ExternalInput")
    rbase_d = nc.dram_tensor("rbase", [B, 1], f32, kind="ExternalInput")
    repl16_d = nc.dram_tensor("repl16", [128, 128], f32, kind="ExternalInput")
    dsel_d = nc.dram_tensor("dsel", [128, 8], f32, kind="ExternalInput")
    out_d = nc.dram_tensor("out", [B, VSH], f32, kind="ExternalOutput")

    # ---- internal DRAM ----
    cand1_in = nc.dram_tensor("cand1_in", [B, K], f32)
    cand1_out = nc.dram_tensor("cand1_out", [B * NCORES, K], f32,
                               addr_space="Shared")
    idxag_in = nc.dram_tensor("idxag_in", [B, K], f32)
    idxag_out = nc.dram_tensor("idxag_out", [B * NCORES, K], f32,
                               addr_space="Shared")
    pag_in = nc.dram_tensor("pag_in", [ASH, B], f32)
    pag_out = nc.dram_tensor("pag_out", [ASH * NCORES, B], f32,
                             addr_space="Shared")
    cand2_in = nc.dram_tensor("cand2_in", [B, K], f32)
    cand2_out = nc.dram_tensor("cand2_out", [B * NCORES, K], f32,
                               addr_space="Shared")
    m2_in = nc.dram_tensor("m2_in", [ASH, B], bf16)
    m2_out = nc.dram_tensor("m2_out", [ASH * NCORES, B], bf16,
                            addr_space="Shared")

    RG = [list(range(NCORES))]

    with tile.TileContext(nc) as tc:
        with (
            tc.tile_pool(name="const", bufs=1) as constp,
            tc.tile_pool(name="small", bufs=1) as smallp,
            tc.tile_pool(name="mbp", bufs=1) as mbp,
        ):
            psA_cm = tc.tile_pool(name="psA", bufs=2, space="PSUM")
            psA = psA_cm.__enter__()

            # ---- startup: keys + first score chunk first, consts after ----
            keysT_sb = constp.tile([128, 8, 128], f32)
            nc.sync.dma_start(out=keysT_sb[:, :, :], in_=keysTt_d[:, :, :])

            rbase = constp.tile([B, 1], f32)
            nc.scalar.dma_start(out=rbase[:, :], in_=rbase_d[:, :])
            repl16 = constp.tile([128, 128], f32)
            nc.scalar.dma_start(out=repl16[:, :], in_=repl16_d[:, :])
            dsel = constp.tile([128, 8], f32)
            nc.scalar.dma_start(out=dsel[:, :], in_=dsel_d[:, :])
            ident = constp.tile([128, 128], f32)
            make_identity(nc, ident[:, :])

            # Mb prefetch pool: NPRE persistent k-slot tiles
            mbpre = [mbp.tile([128, VSH], bf16, tag=f"mb{k}", name=f"mb{k}")
                     for k in range(NPRE)]

            # ---- stage B: score chunks + pipelined per-chunk top-32 ----
            rhsBp_cm = tc.tile_pool(name="rhsB", bufs=3)
            rhsBp = rhsBp_cm.__enter__()
            chkp_cm = tc.tile_pool(name="chk", bufs=3)
            chkp = chkp_cm.__enter__()
            # combined candidate tile: [:, 0] = values, [:, 1] = global idx
            vc = smallp.tile([B, 2, 8, K], f32)
            for n in range(8):
                ps = psA.tile([128, 512], f32, tag="ps", name=f"psB{n}")
                rhs = rhsBp.tile([128, 8, 512], f32, tag="rhs", name=f"rB{n}")
                nc.sync.dma_start(out=rhs[:, 0:4, :], in_=vpTt_d[n, :, 0:4, :])
                nc.scalar.dma_start(out=rhs[:, 4:8, :], in_=vpTt_d[n, :, 4:8, :])
                for k in range(8):
                    nc.tensor.matmul(ps[:, :], keysT_sb[:, k, :], rhs[:, k, :],
                                     start=(k == 0), stop=(k == 7))
                # copy on vector: scalar/sync stay pure DMA issuers in B, so
                # chunk loads enqueue far ahead of the compute
                schunk = chkp.tile([B, 512], f32, tag="schunk", name=f"sch{n}")
                nc.vector.tensor_copy(schunk[:, :], ps[:, :])
                scr = chkp.tile([B, 512], f32, tag="scr", name=f"scr{n}")
                idxn = chkp.tile([B, K], u16, tag="idxn", name=f"idxn{n}")
                for r in range(4):
                    s = schunk if r == 0 else scr
                    nc.vector.max(out=vc[:, 0, n, r * 8:(r + 1) * 8], in_=s[:, :])
                    nc.vector.max_index(out=idxn[:, r * 8:(r + 1) * 8],
                                        in_max=vc[:, 0, n, r * 8:(r + 1) * 8],
                                        in_values=schunk[:, :])
                    nc.vector.match_replace(
                        out=scr[:, :],
                        in_to_replace=vc[:, 0, n, r * 8:(r + 1) * 8],
                        in_values=s[:, :], imm_value=NEG)
                # global index = pos + rank_base + n*512
                nc.vector.tensor_scalar(
                    out=vc[:, 1, n, :], in0=idxn[:, :], scalar1=rbase[:, :],
                    scalar2=float(n * 512), op0=Alu.add, op1=Alu.add)
            # core-level value premerge over the 256 chunk candidates
            vals256f = vc[:, 0, :, :].rearrange("b e k -> b (e k)")
            cmv = smallp.tile([B, K], f32, tag="cmv")
            cms = smallp.tile([B, 8 * K], f32, tag="cms")
            for r in range(4):
                s = vals256f if r == 0 else cms[:, :]
                nc.vector.max(out=cmv[:, r * 8:(r + 1) * 8], in_=s)
                nc.vector.match_replace(
                    out=cms[:, :], in_to_replace=cmv[:, r * 8:(r + 1) * 8],
                    in_values=s, imm_value=NEG)
            nc.sync.dma_start(out=cand1_in[:, :], in_=cmv[:, :])
            nc.gpsimd.collective_compute(
                "AllGather", Alu.bypass, replica_groups=RG,
                ins=[cand1_in.ap().opt()], outs=[cand1_out.ap().opt()])
            chkp_cm.__exit__(None, None, None)
            rhsBp_cm.__exit__(None, None, None)

            # post-B bulk prefetch, all on the scalar queue: sync stays free
            # for the small critical collective bounces and readbacks
            rhsKp_cm = tc.tile_pool(name="rhsK", bufs=2)
            rhsKp = rhsKp_cm.__enter__()
            rhsK = [rhsKp.tile([128, 8, ASH], f32, tag="rhs", name=f"rK{j}")
                    for j in range(4)]
            nc.scalar.dma_start(out=rhsK[0][:, :, :], in_=apT_d[:, 0:8, :])
            nc.scalar.dma_start(out=rhsK[1][:, :, :], in_=apT_d[:, 8:16, :])
            for k in range(0, NPRE):
                nc.scalar.dma_start(out=mbpre[k][:, :],
                                    in_=Mb_d[k * 128:(k + 1) * 128, :])

            # ---- stage C: global value merge -> t32, then index AllGather ----
            mrgp_cm = tc.tile_pool(name="mrg", bufs=1)
            mrgp = mrgp_cm.__enter__()

            def topk32(vals, width, pool, pref):
                """mv [B, 32] = top-32 values of vals [B, width] (descending)."""
                mv = pool.tile([B, K], f32, name=f"{pref}_mv", tag=f"{pref}_mv")
                ms = pool.tile([B, width], f32, name=f"{pref}_ms", tag=f"{pref}_ms")
                for r in range(4):
                    s = vals if r == 0 else ms[:, :]
                    nc.vector.max(out=mv[:, r * 8:(r + 1) * 8], in_=s)
                    nc.vector.match_replace(
                        out=ms[:, :], in_to_replace=mv[:, r * 8:(r + 1) * 8],
                        in_values=s, imm_value=NEG)
                return mv

            gvals = mrgp.tile([B, NCORES, K], f32)
            nc.sync.dma_start(
                out=gvals[:, :, :],
                in_=cand1_out.ap().rearrange("(r b) k -> b r k", r=NCORES, b=B))
            gmv = topk32(gvals[:, :, :].rearrange("b e k -> b (e k)"),
                         NCORES * K, mrgp, "gm")

            # local index extraction under the global threshold
            msk = mrgp.tile([B, 8 * K], u8)
            nc.vector.tensor_scalar(out=msk[:, :], in0=vals256f,
                                    scalar1=gmv[:, K - 1:K], scalar2=None,
                                    op0=Alu.is_ge)
            mi = mrgp.tile([B, 8 * K], f32)
            nc.vector.memset(mi[:, :], -1.0)
            nc.vector.copy_predicated(
                out=mi[:, :], mask=msk[:, :],
                data=vc[:, 1, :, :].rearrange("b e k -> b (e k)"))
            lidx = topk32(mi[:, :], 8 * K, mrgp, "li")

            nc.sync.dma_start(out=idxag_in[:, :], in_=lidx[:, :])
            nc.gpsimd.collective_compute(
                "AllGather", Alu.bypass, replica_groups=RG,
                ins=[idxag_in.ap().opt()], outs=[idxag_out.ap().opt()])
            gidxall = mrgp.tile([B, NCORES, K], f32)
            nc.sync.dma_start(
                out=gidxall[:, :, :],
                in_=idxag_out.ap().rearrange("(r b) k -> b r k", r=NCORES, b=B))
            giv = topk32(gidxall[:, :, :].rearrange("b e k -> b (e k)"),
                         NCORES * K, mrgp, "gi")

            # ---- stage E: build the DGE wrapped index layout on-chip ----
            # idxs16[p', k*8+s0] = giv[16*s0 + p'%16, k].  Spread giv
            # diagonally into R[b, k, s0] (nonzero only when b//16 == s0),
            # then one matmul with the mod-16 replicator sums it into place.
            R = smallp.tile([128, K, 8], f32)
            nc.vector.tensor_tensor(
                out=R[:, :, :],
                in0=giv[:, :].broadcast_to([128, K, 8]),
                in1=dsel[:, None, :].broadcast_to([128, K, 8]),
                op=Alu.mult)
            psI = psA.tile([128, 256], f32, tag="ps", name="psI")
            nc.tensor.matmul(psI[:, :], repl16[:, :],
                             R[:, :, :].rearrange("p k s -> p (k s)"),
                             start=True, stop=True)
            idxs16 = smallp.tile([128, 256], i16)   # 4096 idxs / 16 lanes
            nc.vector.tensor_copy(idxs16[:, :], psI[:, :])
            mrgp_cm.__exit__(None, None, None)

            # SWDGE ring holds 128 descriptors/queue; one gather emits
            # num_idxs/16+1, so split 4096 indices into 4 calls of 1024 on 4
            # queues and tree-reduce each batch of 8 slots as gathers land.
            gatp_cm = tc.tile_pool(name="gat", bufs=2)
            gatp = gatp_cm.__enter__()
            p_chunk = smallp.tile([B, ASH], f32)
            for j in range(4):
                gath = gatp.tile([128, 8, ASH], f32, tag="gath", name=f"gath{j}")
                nc.gpsimd.dma_gather(
                    out_ap=gath[:, :, :], in_ap=Ecol_d.ap(),
                    idxs_ap=idxs16[:, j * 64:(j + 1) * 64],
                    num_idxs=1024, num_idxs_reg=1024, elem_size=ASH)
                a1 = gatp.tile([B, 4, ASH], f32, tag="a1", name=f"a1_{j}", bufs=1)
                nc.vector.tensor_tensor(out=a1[:, :, :], in0=gath[:, 0:4, :],
                                        in1=gath[:, 4:8, :], op=Alu.add)
                a2 = gatp.tile([B, 2, ASH], f32, tag="a2", name=f"a2_{j}", bufs=1)
                nc.vector.tensor_tensor(out=a2[:, :, :], in0=a1[:, 0:2, :],
                                        in1=a1[:, 2:4, :], op=Alu.add)
                if j == 0:
                    nc.vector.tensor_tensor(out=p_chunk[:, :], in0=a2[:, 0, :],
                                            in1=a2[:, 1, :], op=Alu.add)
                else:
                    a3 = gatp.tile([B, ASH], f32, tag="a3", name=f"a3_{j}", bufs=1)
                    nc.vector.tensor_tensor(out=a3[:, :], in0=a2[:, 0, :],
                                            in1=a2[:, 1, :], op=Alu.add)
                    nc.vector.tensor_tensor(out=p_chunk[:, :], in0=p_chunk[:, :],
                                            in1=a3[:, :], op=Alu.add)
            gatp_cm.__exit__(None, None, None)

            # ---- transpose p chunk BEFORE the AllGather, so the gathered
            # result is directly the lhsT layout for stage K ----
            pTp_cm = tc.tile_pool(name="pTp", bufs=1)
            pTp = pTp_cm.__enter__()
            pTc = smallp.tile([128, 4, 128], f32)
            for t in range(4):
                pt = psA.tile([128, 128], f32, tag="ps", name=f"ptJ{t}")
                nc.tensor.transpose(pt[:, :], p_chunk[:, t * 128:(t + 1) * 128],
                                    ident[:, :])
                nc.scalar.copy(pTc[:, t, :], pt[:, :])
            for t in range(4):
                eng = nc.sync if t % 2 == 0 else nc.scalar
                eng.dma_start(out=pag_in[t * 128:(t + 1) * 128, :],
                              in_=pTc[:, t, :])
            nc.gpsimd.collective_compute(
                "AllGather", Alu.bypass, replica_groups=RG,
                ins=[pag_in.ap().opt()], outs=[pag_out.ap().opt()])
            pT = pTp.tile([128, 32, 128], f32)
            nc.sync.dma_start(
                out=pT[:, 0:16, :],
                in_=pag_out.ap()[0:2048, :].rearrange(
                    "(t p) c -> p t c", t=16, p=128))
            nc.gpsimd.dma_start(
                out=pT[:, 16:32, :],
                in_=pag_out.ap()[2048:4096, :].rearrange(
                    "(t p) c -> p t c", t=16, p=128))

            # ---- stage K: scores2 chunk (fp32) ----
            s2 = smallp.tile([B, ASH], f32, tag="s2")
            psK = psA.tile([128, 512], f32, tag="ps", name="psK")
            # ring waits stall only the issuing engine, not the matmul stream
            nc.scalar.dma_start(out=rhsK[2][:, :, :], in_=apT_d[:, 16:24, :])
            nc.sync.dma_start(out=rhsK[3][:, :, :], in_=apT_d[:, 24:32, :])
            for j in range(4):
                for k in range(8):
                    kk = j * 8 + k
                    nc.tensor.matmul(psK[:, :], pT[:, kk, :], rhsK[j][:, k, :],
                                     start=(kk == 0), stop=(kk == 31))
            nc.scalar.copy(s2[:, :], psK[:, :])
            pTp_cm.__exit__(None, None, None)
            rhsKp_cm.__exit__(None, None, None)
            bigp_cm = tc.tile_pool(name="big", bufs=1)
            bigp = bigp_cm.__enter__()

            # ---- stage L/M: local top-32 values, AG, merge -> t32_2 ----
            scr2 = smallp.tile([B, ASH], f32, tag="scr2")
            cand2 = smallp.tile([B, K], f32, tag="c2")
            for r in range(4):
                s = s2 if r == 0 else scr2
                nc.vector.max(out=cand2[:, r * 8:(r + 1) * 8], in_=s[:, :])
                nc.vector.match_replace(
                    out=scr2[:, :], in_to_replace=cand2[:, r * 8:(r + 1) * 8],
                    in_values=s[:, :], imm_value=NEG)
            nc.sync.dma_start(out=cand2_in[:, :], in_=cand2[:, :])
            nc.gpsimd.collective_compute(
                "AllGather", Alu.bypass, replica_groups=RG,
                ins=[cand2_in.ap().opt()], outs=[cand2_out.ap().opt()])
            cands2 = smallp.tile([B, NCORES, K], f32, tag="cs2")
            nc.sync.dma_start(
                out=cands2[:, :, :],
                in_=cand2_out.ap().rearrange("(r b) k -> b r k", r=NCORES, b=B))
            mcand2 = smallp.tile([B, K], f32, tag="mc2")
            mscr2 = smallp.tile([B, NCORES * K], f32, tag="ms2")
            for r in range(4):
                s = (cands2[:, :, :].rearrange("b e k -> b (e k)")
                     if r == 0 else mscr2[:, :])
                nc.vector.max(out=mcand2[:, r * 8:(r + 1) * 8], in_=s)
                nc.vector.match_replace(
                    out=mscr2[:, :], in_to_replace=mcand2[:, r * 8:(r + 1) * 8],
                    in_values=s, imm_value=NEG)

            # ---- stage N/O: mask2, AllGather -> w2 ----
            mask2 = smallp.tile([B, ASH], f32, tag="m2")
            nc.vector.tensor_scalar(
                out=mask2[:, :], in0=s2[:, :], scalar1=mcand2[:, K - 1:K],
                scalar2=None, op0=Alu.is_ge)
            m2Tc = smallp.tile([128, 4, 128], bf16)
            for t in range(4):
                pt = psA.tile([128, 128], f32, tag="ps", name=f"ptP{t}")
                nc.tensor.transpose(pt[:, :], mask2[:, t * 128:(t + 1) * 128],
                                    ident[:, :])
                nc.scalar.copy(m2Tc[:, t, :], pt[:, :])
            for t in range(4):
                eng = nc.sync if t % 2 == 0 else nc.scalar
                eng.dma_start(out=m2_in[t * 128:(t + 1) * 128, :],
                              in_=m2Tc[:, t, :])
            nc.gpsimd.collective_compute(
                "AllGather", Alu.bypass, replica_groups=RG,
                ins=[m2_in.ap().opt()], outs=[m2_out.ap().opt()])

            # ---- stage Q: out chunk = w2 @ M_shard (bf16) ----
            psA_cm.__exit__(None, None, None)
            psQp_cm = tc.tile_pool(name="psQ", bufs=8, space="PSUM")
            psQp = psQp_cm.__enter__()
            rhsQp_cm = tc.tile_pool(name="rhsQ", bufs=4)
            rhsQp = rhsQp_cm.__enter__()

            # pre-issue the first streamed rhs slots before the w2T readback
            rQ = {}
            for k in range(NPRE, min(NPRE + 4, 32)):
                rQ[k] = rhsQp.tile([128, VSH], bf16, tag="rhs", name=f"rQ{k}")
                eng = nc.sync if k % 2 == 0 else nc.scalar
                eng.dma_start(out=rQ[k][:, :],
                              in_=Mb_d[k * 128:(k + 1) * 128, :])

            w2T = bigp.tile([128, 32, 128], bf16, tag="w2T")
            nc.sync.dma_start(
                out=w2T[:, 0:16, :],
                in_=m2_out.ap()[0:2048, :].rearrange(
                    "(t p) c -> p t c", t=16, p=128))
            nc.gpsimd.dma_start(
                out=w2T[:, 16:32, :],
                in_=m2_out.ap()[2048:4096, :].rearrange(
                    "(t p) c -> p t c", t=16, p=128))

            out_sb = bigp.tile([B, VSH], f32, tag="B")
            psQ = [psQp.tile([128, 512], f32, tag="pq", name=f"psQ{n}")
                   for n in range(8)]
            for k in range(32):
                if k < NPRE:
                    rhs = mbpre[k]
                else:
                    if k not in rQ:
                        rQ[k] = rhsQp.tile([128, VSH], bf16, tag="rhs",
                                           name=f"rQ{k}")
                        eng = nc.sync if k % 2 == 0 else nc.scalar
                        eng.dma_start(out=rQ[k][:, :],
                                      in_=Mb_d[k * 128:(k + 1) * 128, :])
                    rhs = rQ[k]
                    if k + 4 < 32:
                        kk = k + 4
                        rQ[kk] = rhsQp.tile([128, VSH], bf16, tag="rhs",
                                            name=f"rQ{kk}")
                        eng = nc.sync if kk % 2 == 0 else nc.scalar
                        eng.dma_start(out=rQ[kk][:, :],
                                      in_=Mb_d[kk * 128:(kk + 1) * 128, :])
                for n in range(8):
                    nc.tensor.matmul(psQ[n][:, :], w2T[:, k, :],
                                     rhs[:, n * 512:(n + 1) * 512],
                                     start=(k == 0), stop=(k == 31))
            # stream each bank out as it completes
            for n in range(8):
                ceng = nc.scalar if n % 2 == 0 else nc.vector
                if n % 2 == 0:
                    ceng.copy(out_sb[:, n * 512:(n + 1) * 512], psQ[n][:, :])
                else:
                    ceng.tensor_copy(out_sb[:, n * 512:(n + 1) * 512],
                                     psQ[n][:, :])
                deng = nc.sync if n % 2 == 0 else nc.scalar
                deng.dma_start(out=out_d[:, n * 512:(n + 1) * 512],
                               in_=out_sb[:, n * 512:(n + 1) * 512])
            psQp_cm.__exit__(None, None, None)
            rhsQp_cm.__exit__(None, None, None)
            bigp_cm.__exit__(None, None, None)

    nc.compile()
    return nc


def get_nc():
    if "nc" not in _CACHE:
        _CACHE["nc"] = _build()
    return _CACHE["nc"]


def make_in_maps(keys, value_proj, clique_encoder, assoc_proj, assoc_mem_value):
    import ml_dtypes
    keysT = np.asarray(keys).T.astype(np.float32)          # [1024, 128]
    keysTt = np.ascontiguousarray(
        keysT.reshape(8, 128, 128).transpose(1, 0, 2))     # [128, 8, 128]
    value_proj = np.asarray(value_proj).astype(np.float32)
    clique_encoder = np.asarray(clique_encoder).astype(np.float32)
    assoc_proj = np.asarray(assoc_proj).astype(np.float32)
    Mb_full = np.asarray(assoc_mem_value).astype(ml_dtypes.bfloat16)
    bb, pp = np.meshgrid(np.arange(128), np.arange(128), indexing="ij")
    repl16 = (bb % 16 == pp % 16).astype(np.float32)
    dsel = (np.arange(128)[:, None] // 16 == np.arange(8)[None, :]).astype(np.float32)
    in_maps = []
    for m in range(NCORES):
        vpT = np.ascontiguousarray(
            value_proj[m * VSH:(m + 1) * VSH, :].T)        # [1024, 4096]
        # [n, p, k, c] so each n-chunk loads with one contiguous-per-partition DMA
        vpTt = np.ascontiguousarray(
            vpT.reshape(8, 128, 8, 512).transpose(2, 1, 0, 3))
        in_maps.append({
            "keysTt": keysTt,
            "vpTt": vpTt,
            "Ecol": np.ascontiguousarray(
                clique_encoder[:, m * ASH:(m + 1) * ASH]),
            "apT": np.ascontiguousarray(
                assoc_proj[m * ASH:(m + 1) * ASH, :].T
                .reshape(32, 128, ASH).transpose(1, 0, 2)),
            "Mb": np.ascontiguousarray(Mb_full[:, m * VSH:(m + 1) * VSH]),
            "rbase": np.full((B, 1), m * VSH, np.float32),
            "repl16": repl16,
            "dsel": dsel,
        })
    return in_maps


def kernel(keys, value_proj, clique_encoder, assoc_proj, assoc_mem_value,
           **run_kwargs):
    from concourse.bass_utils import run_bass_kernel_spmd

    nc = get_nc()
    in_maps = make_in_maps(keys, value_proj, clique_encoder, assoc_proj,
                           assoc_mem_value)
    res = run_bass_kernel_spmd(nc, in_maps, core_ids=list(range(NCORES)),
                               **run_kwargs)
    out = np.concatenate([np.asarray(res.results[m]["out"])
                          for m in range(NCORES)], axis=1)
    _CACHE["last_result"] = res
    return out
